# revision 22
# baseline (speedup 1.0000x reference)
"""Trainium2 Bass kernel for MQA sparse attention (nn_Attention_83356725281353).

Batch-parallel attention across 8 NeuronCores (4 batches each) with
head-sharded projection weights and collective exchanges:

  - wq is sharded 2 heads/core; every core computes q for ALL 128 (b,q)
    rows for its 2 heads (M=128 matmuls, full PE util), plus k_new/v_new
    (wk/wv replicated, also M=128). An AllToAll ships row-block d of the
    per-core q to core d, so each core receives exactly its own 16 rows
    for all 16 heads (8x less wire than an AllGather of the full block).
    k_new/v_new for own rows are selected locally from proj with a
    one-hot select-matmul.
  - The FIRST collective of an execution pays the cross-core launch
    skew (~35us): its barrier starts when the first core triggers and
    ends when the last does, plus ~11us of CC-ring setup.  The q
    AllToAll is therefore triggered as early as possible (wq loads
    before wkv, q projection before kn/vn, qg_in staged on the scalar
    hw queue right after the weight loads) and the whole kv stream
    prefetches underneath it.
  - Engine DMA queues head-of-line block (in-order per queue), so the
    streams are laid out carefully: sync gets kt + pair-1 v, scalar
    gets weights + pair-0 v + bias (+ the early qg_in), and every
    post-collective unpack rides sync AFTER all stream dma_starts have
    been emitted.  gpsimd software-DGE (~7us completion latency) only
    carries collective doorbells and the tiny vn patches.
  - Attention (per-core, 4 batches, kv in 2048-chunks, fully resident
    in SBUF): p^T = kt.T @ qT with kt stationary (full 128-wide PE),
    exp via DVE-add + ACT, o accumulated with a ones column appended to
    v giving softmax denominators for free.
  - wo is sharded 256 output-dims/core; per-pair AllGathers of oT (the
    pair-0 AG overlaps pair-1 attention) let every core compute ALL 128
    rows for its 256-dim slice with M=128 matmuls.  Host concatenates
    dim slices.

Self-contained: hardcodes all shapes; builds/compiles once per process
and runs via run_bass_kernel_spmd on cores 0-7.
"""

import numpy as np

B, Q, DIM, H, HD, KV = 32, 4, 2048, 16, 128, 8192
NCORES = 8
BPC = B // NCORES            # 4 batches per core
BQ = BPC * Q                 # 16 own (b,q) rows per core
RALL = B * Q                 # 128 global rows
ROWS = H * Q                 # 64 attention rows per batch
NPAIR = BPC // 2             # 2 batch-pairs per core
DT = 16                      # dim tiles (DIM/128)
KCH = 2048                   # kv chunk width
NCH = KV // KCH              # 4 chunks per batch
HPC = H // NCORES            # 2 heads per core
DSH = DIM // NCORES          # 256 output dims per core
QW = HPC * HD                # 256 q-projection columns per core
KVW = 2 * HD                 # 256 kn/vn projection columns
VW = HD + 1
RG = [list(range(NCORES))]

_CACHE = {}


def _build():
    import concourse.bass as bass
    import concourse.tile as tile
    from concourse import bacc, mybir, masks

    f32 = mybir.dt.float32
    bf16 = mybir.dt.bfloat16

    nc = bacc.Bacc("TRN2", target_bir_lowering=False, debug=False,
                   num_devices=NCORES)

    # pre-tiled [128, (t, m)]: xT tile t on partitions, rows m in free
    xT = nc.dram_tensor("xT", [128, DT * RALL], bf16,
                        kind="ExternalInput").ap()
    # pre-tiled [128, (t, QW)]: wq for this core's 2 heads (scaled)
    wq_t = nc.dram_tensor("wq_t", [128, DT * QW], bf16,
                          kind="ExternalInput").ap()
    # pre-tiled [128, (t, KVW)]: [wk | wv]
    wkv_t = nc.dram_tensor("wkv_t", [128, DT * KVW], bf16,
                           kind="ExternalInput").ap()
    bproj = nc.dram_tensor("bproj", [1, QW + KVW], bf16,
                           kind="ExternalInput").ap()
    kT = nc.dram_tensor("kT", [BPC, HD, KV], bf16, kind="ExternalInput").ap()
    vv = nc.dram_tensor("vv", [BPC, 128, KV // 128, VW], bf16,
                        kind="ExternalInput").ap()
    # bias[j, p, c, (n t r)]: kv = c*2048 + n*512 + t*128 + p, r = pair-row
    bias = nc.dram_tensor("bias", [NPAIR, 128, NCH, KCH], bf16,
                          kind="ExternalInput").ap()
    wo = nc.dram_tensor("wo", [H * HD, DSH], bf16, kind="ExternalInput").ap()
    bo = nc.dram_tensor("bo", [1, DSH], bf16, kind="ExternalInput").ap()
    ones = nc.dram_tensor("ones", [1, RALL], bf16, kind="ExternalInput").ap()
    sel = nc.dram_tensor("sel", [RALL, BQ], bf16, kind="ExternalInput").ap()
    out = nc.dram_tensor("out", [RALL, DSH], f32, kind="ExternalOutput").ap()

    with tile.TileContext(nc) as tc:
        _body(tc, nc, bass, mybir, masks, xT, wq_t, wkv_t, bproj, kT, vv,
              bias, wo, bo, ones, sel, out)

    nc.compile()
    return nc


def _body(tc, nc, bass, mybir, masks, xT, wq_t, wkv_t, bproj, kT, vv,
          bias, wo, bo, ones, sel, out):
    from contextlib import ExitStack

    f32 = mybir.dt.float32
    bf16 = mybir.dt.bfloat16
    EXP = mybir.ActivationFunctionType.Exp

    with ExitStack() as octx:
        const = octx.enter_context(tc.tile_pool(name="const", bufs=1))
        apool = octx.enter_context(tc.tile_pool(name="a", bufs=6))
        dram = octx.enter_context(tc.tile_pool(name="dram", bufs=1,
                                               space="DRAM"))

        projw_cm = tc.tile_pool(name="projw", bufs=1)
        projw = projw_cm.__enter__()
        ones16 = const.tile([1, RALL], bf16, tag="ones16")
        xT_sb = projw.tile([128, DT * RALL], bf16, tag="xT")
        wq_sb = projw.tile([128, DT * QW], bf16, tag="wq")
        wkv_sb = projw.tile([128, DT * KVW], bf16, tag="wkv")
        bproj_sb = const.tile([1, QW + KVW], bf16, tag="bproj")
        bo_sb = const.tile([1, DSH], bf16, tag="bo")
        sel_sb = const.tile([RALL, BQ], bf16, tag="sel")
        with tc.high_priority():
            # quarter-granularity sems so the proj matmuls pipeline with
            # the transfers instead of waiting on one whole-tensor sem
            for part in range(4):
                t0, t1 = part * (DT // 4), (part + 1) * (DT // 4)
                nc.scalar.dma_start(wq_sb[:, t0 * QW:t1 * QW],
                                    wq_t[:, t0 * QW:t1 * QW])
                nc.scalar.dma_start(xT_sb[:, t0 * RALL:t1 * RALL],
                                    xT[:, t0 * RALL:t1 * RALL])
            nc.scalar.dma_start(ones16[:], ones)
            nc.scalar.dma_start(bproj_sb[:], bproj)
            nc.scalar.dma_start(bo_sb[:], bo)
            nc.scalar.dma_start(sel_sb[:], sel)
            nc.scalar.dma_start(wkv_sb[:], wkv_t)

        ident_f = const.tile([128, 128], f32, tag="idf")
        ident_b = const.tile([128, 128], bf16, tag="idb")
        masks.make_identity(nc, ident_f[:])
        masks.make_identity(nc, ident_b[:])

        proj_sb = const.tile([128, QW + KVW], bf16, tag="proj")
        gq2_sb = const.tile([128, QW], bf16, tag="gq2")
        knT_sb = const.tile([128, BQ], bf16, tag="knT")
        # qT layout: [e, (b, h, q)] col = b*64 + h*4 + q (p-matmul moving)
        qT_sb = const.tile([128, BPC * ROWS], bf16, tag="qT")
        vn_sb = const.tile([BQ, HD], bf16, tag="vn")
        # oT layout: [e=128, (j,h,b2,q)] col = j*128 + h*8 + b2*4 + q
        oT_sb = const.tile([128, BPC * ROWS], bf16, tag="oT")

        # fully-resident kv streams (written by per-chunk dmas; consumers
        # range-depend on exactly their chunk's transfer)
        ktall = const.tile([128, NPAIR * NCH * 2 * KCH], bf16, tag="ktall")
        vvall = const.tile([128, NPAIR * NCH * 2 * 16 * VW], bf16,
                           tag="vvall")
        ball = const.tile([128, NPAIR * NCH * KCH], bf16, tag="ball")

        qg_in = dram.tile([NCORES, BQ, QW], bf16, tag="qgin")
        qg_out = dram.tile([NCORES, BQ, QW], bf16, tag="qgout")
        og_in = [dram.tile([128, 128], bf16, tag=f"ogin{j}", name=f"ogin{j}")
                 for j in range(NPAIR)]
        og_out = [dram.tile([NCORES, 128, 128], bf16, tag=f"ogout{j}",
                            name=f"ogout{j}", addr_space="Shared")
                  for j in range(NPAIR)]

        # ---------------- Phase P: projections + q exchange -----------------
        with (tc.tile_pool(name="qps", bufs=1, space="PSUM") as qps,
              tc.tile_pool(name="wps", bufs=2, space="PSUM") as wps,
              tc.tile_pool(name="ptr", bufs=1, space="PSUM") as ptr):
            for _ in range(10):
                d_ps = wps.tile([128, 128], f32, tag="warm")
                nc.tensor.matmul(d_ps[:], ident_b[:], ident_b[:],
                                 start=True, stop=True)
            proj_ps = qps.tile([128, QW + KVW], f32, tag="projps")
            ones_r = ones16[0:1, :]
            # q for ALL 128 rows of this core's 2 heads
            for t in range(DT):
                nc.tensor.matmul(proj_ps[:, 0:QW],
                                 xT_sb[:, t * RALL:t * RALL + RALL],
                                 wq_sb[:, t * QW:(t + 1) * QW],
                                 start=(t == 0), stop=False)
            nc.tensor.matmul(proj_ps[:, 0:QW], ones_r[0:1, 0:128],
                             bproj_sb[0:1, 0:QW], start=False, stop=True)
            nc.vector.tensor_copy(proj_sb[:, 0:QW], proj_ps[:, 0:QW])
            # ship q row-blocks to their owners ASAP: the AllToAll's trigger
            # time gates the skew barrier (every core fights the same path,
            # so this sets the barrier end, not just ours).  gpsimd SW-DGE
            # has ~7us completion latency but zero queue backlog -- the hw
            # FIFOs would delay qg_in behind megabytes of stream.
            nc.gpsimd.dma_start(
                qg_in[:].rearrange("i r c -> (i r) c"), proj_sb[:, 0:QW])
            nc.gpsimd.collective_compute(
                "AllToAll", mybir.AluOpType.bypass, replica_groups=RG,
                ins=[qg_in.opt()], outs=[qg_out.opt()])
            # kn/vn projections for all rows
            for t in range(DT):
                nc.tensor.matmul(proj_ps[:, QW:QW + KVW],
                                 xT_sb[:, t * RALL:t * RALL + RALL],
                                 wkv_sb[:, t * KVW:(t + 1) * KVW],
                                 start=(t == 0), stop=False)
            nc.tensor.matmul(proj_ps[:, QW:QW + KVW], ones_r[0:1, 0:128],
                             bproj_sb[0:1, QW:QW + KVW], start=False,
                             stop=True)
            nc.vector.tensor_copy(proj_sb[:, QW:QW + KVW],
                                  proj_ps[:, QW:QW + KVW])
            # kn/vn for my rows are local: select from proj_sb with S
            knsel_ps = ptr.tile([128, BQ], f32, tag="knsel")
            nc.tensor.matmul(knsel_ps[:], proj_sb[:, QW:QW + HD],
                             sel_sb[:], start=True, stop=True)
            nc.vector.tensor_copy(knT_sb[:, 0:BQ], knsel_ps[:])
            vn_ps = ptr.tile([BQ, HD], f32, tag="vnsel")
            nc.tensor.matmul(vn_ps[:], sel_sb[:],
                             proj_sb[:, QW + HD:QW + 2 * HD],
                             start=True, stop=True)
            nc.vector.tensor_copy(vn_sb[:], vn_ps[:])

            # ------------ kv stream: all dmas up-front ----------------------
            # sync: kt (8x1MB) + pair-1 v; scalar: pair-0 v + bias (then wo)
            for j in range(NPAIR):
                b0 = 2 * j
                for c in range(NCH):
                    jc = j * NCH + c
                    nc.sync.dma_start(
                        ktall[:, jc * 2 * KCH:(jc + 1) * 2 * KCH]
                        .rearrange("p (b m) -> p b m", b=2),
                        kT[b0:b0 + 2, :, c * KCH:(c + 1) * KCH]
                        .rearrange("b p m -> p b m"))
                    # vv all on sync: keeps the scalar FIFO shallow so
                    # qg_in's completion (the collective doorbell gate)
                    # isn't stuck behind megabytes of stream
                    nc.sync.dma_start(
                        vvall[:, jc * 32 * VW:(jc + 1) * 32 * VW]
                        .rearrange("p (b n e) -> p b n e", b=2, n=16),
                        vv[b0:b0 + 2, :, c * 16:(c + 1) * 16, :]
                        .rearrange("b p n e -> p b n e"))
                    nc.scalar.dma_start(
                        ball[:, jc * KCH:(jc + 1) * KCH], bias[j][:, c, :])

            # keep the PE from sleeping through the collective wait, paced
            # by kt chunk arrivals, gated after proj (rhs dep)
            for w in range(NPAIR * NCH):
                d_ps = wps.tile([128, 128], f32, tag="warm2")
                nc.tensor.matmul(d_ps[:], ktall[:, w * 2 * KCH:w * 2 * KCH + 128],
                                 proj_sb[:, 0:128], start=True, stop=True)

            # ------------ receive q, assemble qT ----------------------------
            # unpack rides sync AFTER all stream dmas (in-order queues!)
            nc.sync.dma_start(gq2_sb[:],
                              qg_out[:].rearrange("i r c -> (i r) c"))
            # gq2 row (i, b, q) holds q[e] for heads (2i, 2i+1) of my row
            # (b, q); transpose each head-half and scatter into qT
            for h2 in range(HPC):
                tr = ptr.tile([128, 128], bf16, tag="qtr", name="qtr")
                nc.tensor.transpose(tr[:], gq2_sb[:, h2 * HD:(h2 + 1) * HD],
                                    ident_b[:])
                nc.vector.tensor_copy(
                    qT_sb[:].rearrange("p (b i h2 q) -> p h2 b i q", b=BPC,
                                       i=NCORES, h2=HPC)[:, h2],
                    tr[:].rearrange("p (i b q) -> p b i q", i=NCORES, b=BPC))

        # projection weights are dead past phase P; close their pool and
        # reuse the SBUF for the output-side tiles
        projw_cm.__exit__(None, None, None)
        late = octx.enter_context(tc.tile_pool(name="late", bufs=1))
        ogs_sb = late.tile([128, NCORES * 128], bf16, tag="ogs")
        # oh layout: [e, (h, i, j, b2, q)] -> col h*128 + global row
        oh_sb = late.tile([128, H * RALL], bf16, tag="oh")
        wo_sb = late.tile([128, H * DSH], bf16, tag="wo")
        nc.scalar.dma_start(
            wo_sb[:].rearrange("p (h d) -> p h d", h=H),
            wo.rearrange("(h p) d -> p h d", p=HD))

        # ---------------- Phase A: attention, per batch-pair ---------------
        with (tc.tile_pool(name="pps", bufs=5, space="PSUM") as pps,
              tc.tile_pool(name="tps", bufs=1, space="PSUM") as tps,
              tc.tile_pool(name="ops", bufs=2, space="PSUM") as ops):
            for j in range(NPAIR):
                b0, b1 = 2 * j, 2 * j + 1
                o_ps = ops.tile([128, VW], f32, tag="o")
                for c in range(NCH):
                    jc = j * NCH + c
                    ktp = ktall[:, jc * 2 * KCH:(jc + 1) * 2 * KCH]
                    kt0 = ktp[:, 0:KCH]
                    kt1 = ktp[:, KCH:2 * KCH]
                    vp = vvall[:, jc * 32 * VW:(jc + 1) * 32 * VW]
                    v0 = vp[:, 0:16 * VW]
                    v1 = vp[:, 16 * VW:32 * VW]
                    bias_sb = ball[:, jc * KCH:(jc + 1) * KCH]
                    if c == NCH - 1:
                        nc.vector.tensor_copy(kt0[:, KCH - 4:KCH],
                                              knT_sb[:, b0 * 4:b0 * 4 + 4])
                        nc.vector.tensor_copy(kt1[:, KCH - 4:KCH],
                                              knT_sb[:, b1 * 4:b1 * 4 + 4])
                        nc.gpsimd.dma_start(
                            v0[124:128, 15 * VW:15 * VW + HD],
                            vn_sb[b0 * 4:b0 * 4 + 4, :])
                        nc.gpsimd.dma_start(
                            v1[124:128, 15 * VW:15 * VW + HD],
                            vn_sb[b1 * 4:b1 * 4 + 4, :])
                    for n in range(4):
                        p_ps = pps.tile([128, 512], f32, tag="p")
                        for t in range(4):
                            ko = (n * 4 + t) * 128
                            nc.tensor.matmul(
                                p_ps[:, t * 128:t * 128 + ROWS],
                                kt0[:, ko:ko + 128],
                                qT_sb[:, b0 * ROWS:(b0 + 1) * ROWS],
                                start=True, stop=True)
                            nc.tensor.matmul(
                                p_ps[:, t * 128 + ROWS:(t + 1) * 128],
                                kt1[:, ko:ko + 128],
                                qT_sb[:, b1 * ROWS:(b1 + 1) * ROWS],
                                start=True, stop=True)
                        e_sb = apool.tile([128, 512], mybir.dt.float16,
                                          tag="e")
                        nc.vector.tensor_tensor(
                            e_sb[:], p_ps[:], bias_sb[:, n * 512:(n + 1) * 512],
                            op=mybir.AluOpType.add)
                        a_bf = apool.tile([128, 512], bf16, tag="abf")
                        nc.scalar.activation(a_bf[:], e_sb[:], EXP)
                        for t in range(4):
                            kvt = c * 16 + n * 4 + t
                            first, last = (kvt == 0), (kvt == 63)
                            vo = (n * 4 + t) * VW
                            nc.tensor.matmul(
                                o_ps[0:ROWS, :],
                                a_bf[:, t * 128:t * 128 + ROWS],
                                v0[:, vo:vo + VW], start=first, stop=last)
                            nc.tensor.matmul(
                                o_ps[ROWS:128, :],
                                a_bf[:, t * 128 + ROWS:(t + 1) * 128],
                                v1[:, vo:vo + VW], start=first, stop=last,
                                tile_position=(0, 64))
                        if j == NPAIR - 1 and c == NCH - 1:
                            # keep the PE activity window busy through the
                            # DVE/ACT-paced drain of the last chunk
                            for _ in range(2):
                                d_ps = pps.tile([128, 512], f32, tag="p")
                                nc.tensor.matmul(d_ps[:, :], ident_b[:],
                                                 bias_sb[:, 0:512],
                                                 start=True, stop=True)
                _finalize_pair(tc, nc, mybir, apool, tps, j, o_ps, oT_sb,
                               ident_f)
                # ship this pair's oT immediately: AG for pair 0 overlaps
                # pair 1's attention; only the small pair-1 AG is a tail.
                # pack/unpack ride the sync hw queue (already past its
                # stream entries); doorbells on gpsimd.
                nc.sync.dma_start(og_in[j][:],
                                  oT_sb[:, j * 128:(j + 1) * 128])
                nc.gpsimd.collective_compute(
                    "AllGather", mybir.AluOpType.bypass, replica_groups=RG,
                    ins=[og_in[j].opt()], outs=[og_out[j].opt()])
                # unpack on scalar (idle after the exps): on sync it would
                # head-block og_in1 behind the AG0 wait
                nc.scalar.dma_start(
                    ogs_sb[:].rearrange("p (i c) -> p i c", i=NCORES),
                    og_out[j][:].rearrange("i p c -> p i c"))
                # oh[e, (h, i, j, b2, q)] = ogs[e, (i, h, b2, q)]
                nc.vector.tensor_copy(
                    oh_sb[:].rearrange(
                        "p (h i j r) -> p h i j r", h=H, i=NCORES,
                        j=NPAIR)[:, :, :, j, :],
                    ogs_sb[:].rearrange(
                        "p (i h r) -> p h i r", i=NCORES, h=H))
                if j == NPAIR - 1:
                    for _ in range(3):
                        d_ps = pps.tile([128, 512], f32, tag="p")
                        nc.tensor.matmul(d_ps[:, :], ident_b[:],
                                         ball[:, 0:512],
                                         start=True, stop=True)

        # ---------------- Phase O: output projection ------------------------
        with tc.tile_pool(name="outps", bufs=2, space="PSUM") as outps:
            # keep PE warm across the last collective wait
            for _ in range(4):
                d_ps = outps.tile([128, 128], f32, tag="warm")
                nc.tensor.matmul(d_ps[:], ident_b[:], ident_b[:],
                                 start=True, stop=True)
            out_ps = outps.tile([RALL, DSH], f32, tag="out")
            for h in range(H):
                nc.tensor.matmul(out_ps[:], oh_sb[:, h * 128:(h + 1) * 128],
                                 wo_sb[:, h * DSH:(h + 1) * DSH],
                                 start=(h == 0), stop=False)
            ones_r = ones16[0:1, :]
            nc.tensor.matmul(out_ps[:], ones_r[0:1, 0:RALL], bo_sb[0:1, :],
                             start=False, stop=True)
            out_sb = late.tile([RALL, DSH], f32, tag="osb")
            nc.vector.tensor_copy(out_sb[:], out_ps[:])
            nc.sync.dma_start(out, out_sb[:])


def _finalize_pair(tc, nc, mybir, apool, tps, j, o_ps, oT_sb, ident_f):
    f32 = mybir.dt.float32
    recip = apool.tile([128, 1], f32, tag="recip")
    nc.vector.reciprocal(recip[:], o_ps[:, HD:HD + 1])
    o_sb = apool.tile([128, HD], f32, tag="osb2")
    nc.vector.tensor_scalar_mul(o_sb[:], o_ps[:, 0:HD], recip[:])
    tr = tps.tile([128, 128], f32, tag="tr")
    nc.tensor.transpose(tr[:], o_sb[:], ident_f[:])
    # oT col j*128 + h*8 + b2*4 + q <- tr col b2*64 + h*4 + q
    oT_4d = oT_sb[:].rearrange("p (j h b2 q) -> p j h b2 q", j=NPAIR, h=H,
                               b2=2)
    nc.vector.tensor_copy(
        oT_4d[:, j],
        tr[:].rearrange("p (b2 h q) -> p h b2 q", b2=2, h=H))


def _get_nc():
    if "nc" not in _CACHE:
        _CACHE["nc"] = _build()
    return _CACHE["nc"]


def kernel(x, attn_bias, cache_k, cache_v, wq, bq, wk, bk, wv, bv, wo, bo):
    import ml_dtypes
    from concourse.bass_utils import run_bass_kernel_spmd

    nc = _get_nc()
    scale = np.float32(1.0 / np.sqrt(HD))
    bf = ml_dtypes.bfloat16

    x = np.asarray(x, np.float32)
    # pre-tiled [128, (t, m)]: per t, tile = x.T[t*128:(t+1)*128, :]
    xTf = np.ascontiguousarray(x.reshape(RALL, DIM).T)          # [DIM, 128]
    xT_tiled = np.ascontiguousarray(
        xTf.reshape(DT, 128, RALL).transpose(1, 0, 2).reshape(128, DT * RALL)
    ).astype(bf)
    wq_s = np.asarray(wq, np.float32) * scale          # [DIM, H, HD]
    bq_s = np.asarray(bq, np.float32) * scale          # [H, HD]
    wk_f = np.asarray(wk, np.float32)
    wv_f = np.asarray(wv, np.float32)
    bk_f = np.asarray(bk, np.float32)
    bv_f = np.asarray(bv, np.float32)
    kTh = np.ascontiguousarray(
        np.roll(np.asarray(cache_k, np.float32), -Q, axis=1)
        .transpose(0, 2, 1)).astype(bf)
    vr0 = np.roll(np.asarray(cache_v, np.float32), -Q, axis=1)
    # [B, KV, HD] -> [B, 128, KV/128, HD+1]: per-partition-contiguous runs,
    # last column = 1.0 so the o-matmul accumulates softmax denominators
    vrh4 = vr0.reshape(B, KV // 128, 128, HD).transpose(0, 2, 1, 3)
    vrh = np.ones((B, 128, KV // 128, HD + 1), np.float32)
    vrh[..., :HD] = vrh4
    vrh = np.ascontiguousarray(vrh).astype(bf)
    # bias -> [pair, p, c, (n t r)] with kv = c*2048 + n*512 + t*128 + p
    ab = np.asarray(attn_bias, np.float32).reshape(B // 2, 2, ROWS, KV)
    abP = ab.transpose(0, 3, 1, 2).reshape(B // 2, KV, 2 * ROWS)
    biasP = np.ascontiguousarray(
        abP.reshape(B // 2, NCH, 4, 4, 128, 2 * ROWS)
        .transpose(0, 4, 1, 2, 3, 5)
        .reshape(B // 2, 128, NCH, KCH)).astype(bf)
    wo_f = np.asarray(wo, np.float32).reshape(H * HD, DIM)
    bo_f = np.asarray(bo, np.float32)
    selm = np.eye(RALL, dtype=np.float32).astype(bf)

    def tile16(w):                                     # [DIM, W] -> pre-tiled
        W = w.shape[1]
        return np.ascontiguousarray(
            w.reshape(DT, 128, W).transpose(1, 0, 2).reshape(128, DT * W)
        ).astype(bf)

    wkv = np.concatenate([wk_f, wv_f], axis=1)         # [DIM, 256]
    wkv_tiled = tile16(wkv)

    in_maps = []
    for c in range(NCORES):
        wq_c = wq_s[:, 2 * c:2 * c + 2, :].reshape(DIM, QW)
        bproj_c = np.concatenate(
            [bq_s[2 * c:2 * c + 2].reshape(QW), bk_f, bv_f])
        in_maps.append({
            "xT": xT_tiled,
            "wq_t": tile16(wq_c),
            "wkv_t": wkv_tiled,
            "bproj": np.ascontiguousarray(
                bproj_c.reshape(1, QW + KVW)).astype(bf),
            "kT": np.ascontiguousarray(kTh[c * BPC:(c + 1) * BPC]),
            "vv": np.ascontiguousarray(vrh[c * BPC:(c + 1) * BPC]),
            "bias": np.ascontiguousarray(biasP[NPAIR * c:NPAIR * (c + 1)]),
            "wo": np.ascontiguousarray(
                wo_f[:, c * DSH:(c + 1) * DSH]).astype(bf),
            "bo": np.ascontiguousarray(
                bo_f[c * DSH:(c + 1) * DSH].reshape(1, DSH)).astype(bf),
            "ones": np.ones((1, RALL), bf),
            "sel": np.ascontiguousarray(selm[:, c * BQ:(c + 1) * BQ]),
        })

    res = run_bass_kernel_spmd(nc, in_maps, core_ids=list(range(NCORES)))
    _CACHE["last_result"] = res
    outs = [res.results[c]["out"] for c in range(NCORES)]
    full = np.concatenate(outs, axis=1)                # [128, DIM]
    return full.reshape(B, Q, DIM).astype(np.float32)
